# revision 2
# baseline (speedup 1.0000x reference)
"""MACE edge-message block on 8 Trainium2 NeuronCores (Bass/Tile) — v4.

Learned from v1-v3 traces:
  - The whole core clock-gates on PE activity (K=4/8 -> half clock for all
    engines ~46% of the time in v1); keeping PE dense matters more than
    shaving single ops.
  - 0-stride repeat APs drop DVE to 1x mode: never use them.
  - Scattered feature-major DMA (1KB runs at 25.6KB stride) clogs HWDGE
    rings in steady state: host packs the pre-gathered sender features AND
    the output in chunk-major [NCHUNK, 128, 4, C] blocks so every DMA is
    4KB-contiguous per partition.
  - HWDGE rings are FIFO per engine: loads prefetched one chunk ahead,
    G+store on sync ring, ef+B on scalar ring.
Structure vs v1: no dma_gather (host pre-gather), w00p/w01p consumed from
PSUM (drops t00/t01 copies), d-sum folded into PSUM-accumulated matmuls
(drops the dr/dr2 adds), single packed store per chunk.
"""

import numpy as np
import ml_dtypes
from contextlib import ExitStack

N_NODES = 20000
N_EDGES = 100000
MUL = 128
R = 8
H = 64
NCORES = 8
ESH = N_EDGES // NCORES          # 12500 real edges per core
C = 512                          # edge chunk (free dim)
EP = 12800                       # padded edges per core (25 * 512)
NCHUNK = EP // C
SILU_NORM = 1.6790390826
INV_SQRT3 = 1.0 / np.sqrt(3.0)
PW_0E = np.sqrt(0.5)
PW_1O = np.sqrt(1.5)
BF16 = ml_dtypes.bfloat16

# ---- tuning flags ----
H3S_ENG = "dve"      # h3s = h3 * sh0
DT_ENG = "dve"       # dt = G[:,1:4]*B[:,1:4]
DP_ACCUM = True      # dp = sum_i MM(wupv, dt_i) via PSUM accumulation
                     # (False: dr/dr2 adds on DVE then one MM)
DIRECT_PSUM = True   # a0/m01 read w00p/w01p straight from PSUM
N_OUT_ACT = 2        # how many of the 4 output copies go on ACT (rest DVE)
PM_BUFS = 6
PREFETCH = True      # emit chunk j+1 loads before chunk j store
N_FILLER = 0         # dummy LDWEIGHTS per chunk to hold the PE clock gate open

_CACHE = {}


def _build_program(reps=1):
    import concourse.bass as bass
    import concourse.tile as tile
    from concourse import bacc, mybir

    bf = mybir.dt.bfloat16
    f32 = mybir.dt.float32
    Silu = mybir.ActivationFunctionType.Silu

    nc = bacc.Bacc(
        "TRN2",
        target_bir_lowering=False,
        debug=False,
        num_devices=NCORES,
    )

    gt = nc.dram_tensor("gt", [NCHUNK, 128, 4, C], bf, kind="ExternalInput")
    eft = nc.dram_tensor("eft", [R, EP], bf, kind="ExternalInput")
    eat = nc.dram_tensor("eat", [4, EP], bf, kind="ExternalInput")
    w0 = nc.dram_tensor("w0", [R, H], bf, kind="ExternalInput")
    w1 = nc.dram_tensor("w1", [H, H], bf, kind="ExternalInput")
    w2 = nc.dram_tensor("w2", [H, H], bf, kind="ExternalInput")
    w3 = nc.dram_tensor("w3", [H, 512], bf, kind="ExternalInput")
    wup = nc.dram_tensor("wup", [128, 256], bf, kind="ExternalInput")
    wout = nc.dram_tensor("wout", [128, 512], bf, kind="ExternalInput")
    outt = nc.dram_tensor("outt", [NCHUNK, 128, 4, C], bf, kind="ExternalOutput")

    with tile.TileContext(nc) as tc, ExitStack() as ctx:
        eng = {"gp": nc.gpsimd, "dve": nc.vector}
        const = ctx.enter_context(tc.tile_pool(name="const", bufs=1))

        def load_const(dram, shape, dt_, name):
            t = const.tile(shape, dt_, name=name, tag=name)
            nc.sync.dma_start(t[:], dram[:])
            return t

        w0s = load_const(w0, [R, H], bf, "w0s")
        w1s = load_const(w1, [H, H], bf, "w1s")
        w2s = load_const(w2, [H, H], bf, "w2s")
        w3s = load_const(w3, [H, 512], bf, "w3s")
        wups = load_const(wup, [128, 256], bf, "wups")
        wouts = load_const(wout, [128, 512], bf, "wouts")

        gp = ctx.enter_context(tc.tile_pool(name="gp", bufs=4))
        bp = ctx.enter_context(tc.tile_pool(name="bp", bufs=4))
        ep = ctx.enter_context(tc.tile_pool(name="ep", bufs=4))
        sb = ctx.enter_context(tc.tile_pool(name="sb", bufs=4))
        ob = ctx.enter_context(tc.tile_pool(name="ob", bufs=3))
        pm = ctx.enter_context(tc.tile_pool(name="pm", bufs=PM_BUFS, space="PSUM"))
        po = ctx.enter_context(tc.tile_pool(name="po", bufs=2, space="PSUM"))

        def filler(ap):
            # dependency-tied dummy weight load: keeps the PE activity
            # monitor busy so the clock gate stays at 8/8
            if N_FILLER > 0:
                nc.tensor.ldweights(ap)

        def emit_loads(j):
            c0 = j * C
            G = gp.tile([128, 4, C], bf, tag="G", name=f"G{j}")
            nc.sync.dma_start(
                G[:], bass.AP(gt, j * 128 * 4 * C, [[4 * C, 128], [C, 4], [1, C]])
            )
            ef = ep.tile([R, C], bf, tag="ef", name=f"ef{j}")
            nc.scalar.dma_start(ef[:], eft[:, c0:c0 + C])
            # per-edge SH scalars, partition-broadcast: B[p,k,e] = eat[k, c0+e]
            B = bp.tile([128, 4, C], bf, tag="B", name=f"B{j}")
            nc.scalar.dma_start(B[:], bass.AP(eat, c0, [[0, 128], [EP, 4], [1, C]]))
            return G, ef, B

        rep_cm = tc.For_i(0, reps, 1) if reps > 1 else None
        if rep_cm is not None:
            rep_cm.__enter__()

        pending = emit_loads(0) if PREFETCH else None
        for j in range(NCHUNK):
            if PREFETCH:
                G, ef, B = pending
                if j + 1 < NCHUNK:
                    pending = emit_loads(j + 1)
            else:
                G, ef, B = emit_loads(j)

            # ---- radial MLP ----
            h1p = pm.tile([H, C], f32, tag="mm")
            nc.tensor.matmul(h1p[:], w0s[:], ef[:], start=True, stop=True)
            h1 = sb.tile([H, C], bf, tag="h1")
            nc.scalar.activation(h1[:], h1p[:], Silu)
            h2p = pm.tile([H, C], f32, tag="mm")
            nc.tensor.matmul(h2p[:], w1s[:], h1[:], start=True, stop=True)
            if N_FILLER >= 1:
                filler(h1[:, 0:128])
            h2 = sb.tile([H, C], bf, tag="h2")
            nc.scalar.activation(h2[:], h2p[:], Silu)
            h3p = pm.tile([H, C], f32, tag="mm")
            nc.tensor.matmul(h3p[:], w2s[:], h2[:], start=True, stop=True)
            if N_FILLER >= 2:
                filler(h2[:, 0:128])
            h3 = sb.tile([H, C], bf, tag="h3")
            nc.scalar.activation(h3[:], h3p[:], Silu)
            # h3 pre-scaled by sh0 for the blocks whose path carries sh0
            h3s = sb.tile([H, C], bf, tag="h3s")
            eng[H3S_ENG].tensor_mul(h3s[:], h3[:], B[0:H, 0, :])

            # ---- up-projection of gathered sender features ----
            ssp = pm.tile([128, C], f32, tag="mm")
            nc.tensor.matmul(ssp[:], wups[:, 0:128], G[:, 0, :], start=True, stop=True)
            ss = sb.tile([128, C], bf, tag="ss")
            nc.scalar.copy(ss[:], ssp[:])
            vsp = [pm.tile([128, C], f32, tag="mm", name=f"vsp{i}") for i in range(3)]
            for i in range(3):
                nc.tensor.matmul(
                    vsp[i][:], wups[:, 128:256], G[:, 1 + i, :], start=True, stop=True
                )

            # ---- tensor-product weights (sh0 pre-folded into blocks 00/10) ----
            if N_FILLER >= 3:
                filler(h3[:, 0:128])
            w00p = pm.tile([128, C], f32, tag="mm")
            nc.tensor.matmul(w00p[:], w3s[:, 0:128], h3s[:], start=True, stop=True)
            w01p = pm.tile([128, C], f32, tag="mm")
            nc.tensor.matmul(w01p[:], w3s[:, 128:256], h3[:], start=True, stop=True)
            w10p = pm.tile([128, C], f32, tag="mm")
            nc.tensor.matmul(w10p[:], w3s[:, 256:384], h3s[:], start=True, stop=True)
            w11p = pm.tile([128, C], f32, tag="mm")
            nc.tensor.matmul(w11p[:], w3s[:, 384:512], h3[:], start=True, stop=True)
            if not DIRECT_PSUM:
                t00 = sb.tile([128, C], bf, tag="t00")
                nc.scalar.copy(t00[:], w00p[:])
                t01 = sb.tile([128, C], bf, tag="t01")
                nc.scalar.copy(t01[:], w01p[:])
            t10 = sb.tile([128, C], bf, tag="t10")
            nc.scalar.copy(t10[:], w10p[:])
            t11 = sb.tile([128, C], bf, tag="t11")
            nc.scalar.copy(t11[:], w11p[:])

            # ---- CG tensor product (elementwise, feature-major) ----
            if N_FILLER >= 4:
                filler(ss[:, 0:128])
            if N_FILLER >= 5:
                filler(t10[:, 0:128])
            a0 = sb.tile([128, C], bf, tag="a0")
            nc.vector.tensor_mul(a0[:], w00p[:] if DIRECT_PSUM else t00[:], ss[:])
            m01 = sb.tile([128, C], bf, tag="m01")
            nc.vector.tensor_mul(m01[:], w01p[:] if DIRECT_PSUM else t01[:], ss[:])
            a1t = [sb.tile([128, C], bf, tag=f"a1_{i}", name=f"a1_{i}")
                   for i in range(3)]
            for i in range(3):
                nc.vector.tensor_mul(a1t[i][:], m01[:], B[:, 1 + i, :])
            q = [sb.tile([128, C], bf, tag=f"q{i}", name=f"q{i}") for i in range(3)]
            for i in range(3):
                nc.vector.tensor_mul(q[i][:], t10[:], vsp[i][:])
            # d = sum_i v_i * sh1_i (raw gathered v), projected by wup_v
            dt_ = sb.tile([128, 3, C], bf, tag="dt")
            eng[DT_ENG].tensor_mul(dt_[:], G[:, 1:4, :], B[:, 1:4, :])
            dp = pm.tile([128, C], f32, tag="mm")
            if DP_ACCUM:
                for i in range(3):
                    nc.tensor.matmul(dp[:], wups[:, 128:256], dt_[:, i, :],
                                     start=(i == 0), stop=(i == 2))
            else:
                dr = sb.tile([128, C], bf, tag="dr")
                nc.vector.tensor_add(dr[:], dt_[:, 0, :], dt_[:, 1, :])
                dr2 = sb.tile([128, C], bf, tag="dr2")
                nc.vector.tensor_add(dr2[:], dr[:], dt_[:, 2, :])
                nc.tensor.matmul(dp[:], wups[:, 128:256], dr2[:],
                                 start=True, stop=True)
            b0 = sb.tile([128, C], bf, tag="b0")
            nc.vector.tensor_mul(b0[:], t11[:], dp[:])

            # ---- output linears (K split 128+128, PSUM accumulate) ----
            osb = ob.tile([128, 4, C], bf, tag="osb")
            for i in range(3):
                ovp = po.tile([128, C], f32, tag="oo", name=f"ovp{i}")
                nc.tensor.matmul(
                    ovp[:], wouts[:, 256:384], a1t[i][:], start=True, stop=False
                )
                nc.tensor.matmul(
                    ovp[:], wouts[:, 384:512], q[i][:], start=False, stop=True
                )
                if N_FILLER >= 6 + i:
                    filler(q[i][:, 0:128])
                if i < N_OUT_ACT:
                    nc.scalar.copy(osb[:, 1 + i, :], ovp[:])
                else:
                    nc.vector.tensor_copy(osb[:, 1 + i, :], ovp[:])
            osp = po.tile([128, C], f32, tag="oo")
            nc.tensor.matmul(osp[:], wouts[:, 0:128], a0[:], start=True, stop=False)
            nc.tensor.matmul(osp[:], wouts[:, 128:256], b0[:], start=False, stop=True)
            if N_OUT_ACT > 3:
                nc.scalar.copy(osb[:, 0, :], osp[:])
            else:
                nc.vector.tensor_copy(osb[:, 0, :], osp[:])
            nc.sync.dma_start(
                bass.AP(outt, j * 128 * 4 * C, [[4 * C, 128], [C, 4], [1, C]]),
                osb[:],
            )
        if rep_cm is not None:
            rep_cm.__exit__(None, None, None)

    nc.compile()
    return nc


def _get_program():
    if "nc" not in _CACHE:
        _CACHE["nc"] = _build_program()
    return _CACHE["nc"]


def _prep_static(W_up_s, W_up_v, mlp_w0, mlp_w1, mlp_w2, mlp_w3,
                 W_out_s, W_out_v):
    """Host-side weight prep (shared across cores)."""
    w0 = np.asarray(mlp_w0, np.float32) / np.sqrt(R)
    w1 = np.asarray(mlp_w1, np.float32) / np.sqrt(H) * SILU_NORM
    w2 = np.asarray(mlp_w2, np.float32) / np.sqrt(H) * SILU_NORM
    w3 = np.asarray(mlp_w3, np.float32) / np.sqrt(H) * SILU_NORM

    wup = np.concatenate(
        [np.asarray(W_up_s, np.float32), np.asarray(W_up_v, np.float32)], axis=1
    ) / np.sqrt(MUL)

    wos = np.asarray(W_out_s, np.float32) / np.sqrt(2 * MUL)
    wov = np.asarray(W_out_v, np.float32) / np.sqrt(2 * MUL)
    wos_top = wos[:MUL] * PW_0E
    wos_bot = wos[MUL:] * (PW_0E * INV_SQRT3)
    wov_sc = wov * (PW_1O * INV_SQRT3)
    wout = np.concatenate(
        [wos_top, wos_bot, wov_sc[:MUL], wov_sc[MUL:]], axis=1
    )

    return dict(
        w0=np.ascontiguousarray(w0).astype(BF16),
        w1=np.ascontiguousarray(w1).astype(BF16),
        w2=np.ascontiguousarray(w2).astype(BF16),
        w3=np.ascontiguousarray(w3).astype(BF16),
        wup=np.ascontiguousarray(wup).astype(BF16),
        wout=np.ascontiguousarray(wout).astype(BF16),
    )


def _prep_core(k, nft, sender, edge_attrs, edge_feats):
    """Per-core inputs: pre-gathered sender features packed chunk-major
    [NCHUNK, 128, 4, C] + edge tensors in feature-major [feat, EP]."""
    lo, hi = k * ESH, (k + 1) * ESH
    ef = np.zeros((EP, R), np.float32)
    ef[:ESH] = edge_feats[lo:hi]
    ea = np.zeros((EP, 4), np.float32)
    ea[:ESH] = edge_attrs[lo:hi]
    g = np.zeros((EP, 512), BF16)
    g[:ESH] = nft[sender[lo:hi]]
    # [EP, (k,p)] -> [NCHUNK, p, k, e]
    g3 = np.ascontiguousarray(
        g.reshape(NCHUNK, C, 4, 128).transpose(0, 3, 2, 1)
    )
    return dict(
        eft=np.ascontiguousarray(ef.T).astype(BF16),
        eat=np.ascontiguousarray(ea.T).astype(BF16),
        gt=g3,
    )


def _prep_all(node_feats, edge_attrs, edge_feats, edge_index,
              W_up_s, W_up_v, mlp_w0, mlp_w1, mlp_w2, mlp_w3,
              W_out_s, W_out_v):
    static = _prep_static(W_up_s, W_up_v, mlp_w0, mlp_w1, mlp_w2,
                          mlp_w3, W_out_s, W_out_v)
    nf = np.asarray(node_feats, np.float32)
    s = nf[:, :MUL]
    v = nf[:, MUL:].reshape(N_NODES, MUL, 3)
    nft = np.concatenate(
        [s, v[:, :, 0], v[:, :, 1], v[:, :, 2]], axis=1
    ).astype(BF16)
    sender = np.asarray(edge_index)[0]
    ea = np.asarray(edge_attrs, np.float32)
    ef = np.asarray(edge_feats, np.float32)
    in_maps = []
    for k in range(NCORES):
        m = dict(static)
        m.update(_prep_core(k, nft, sender, ea, ef))
        in_maps.append(m)
    return in_maps


def _unscramble(ot, out_slice):
    """[NCHUNK, 128, 4, C] core output -> [ESH, 512] rows of the result."""
    arr = np.asarray(ot, np.float32).transpose(0, 3, 2, 1).reshape(EP, 512)
    arr = arr[:ESH]
    out_slice[:, :MUL] = arr[:, :MUL]
    out_slice[:, MUL:] = (
        arr[:, MUL:].reshape(ESH, 3, MUL).transpose(0, 2, 1).reshape(ESH, 3 * MUL)
    )


def kernel(node_feats, edge_attrs, edge_feats, edge_index,
           W_up_s, W_up_v, mlp_w0, mlp_w1, mlp_w2, mlp_w3,
           W_out_s, W_out_v, _want_results=False, _trace=False):
    from concourse.bass_utils import run_bass_kernel_spmd

    nc = _get_program()
    in_maps = _prep_all(node_feats, edge_attrs, edge_feats, edge_index,
                        W_up_s, W_up_v, mlp_w0, mlp_w1, mlp_w2, mlp_w3,
                        W_out_s, W_out_v)

    res = run_bass_kernel_spmd(
        nc, in_maps, core_ids=list(range(NCORES)), trace=_trace
    )

    out = np.empty((N_EDGES, 4 * MUL), np.float32)
    for k in range(NCORES):
        lo, hi = k * ESH, (k + 1) * ESH
        _unscramble(res.results[k]["outt"], out[lo:hi])
    if _want_results:
        return out, res
    return out
